# revision 1
# baseline (speedup 1.0000x reference)
"""Causal self-attention on 8 TRN2 NeuronCores.

Problem: x[4, 2048, 1024], qkv_w[1024, 3072], o_w[1024, 1024] (f32).
Sharding: core c = (batch b = c // 2, head-group g = c % 2 of 8 heads).
Each core computes qkv projection for its (batch, 8 heads), causal
attention, and a partial o_proj ([2048, 1024], f32).  Host sums the two
head-group partials per batch (the "all-reduce") and adds o_b.

Device-side layout choices:
  - All matmuls in bf16 (f32 PSUM accumulate); host pre-casts inputs.
  - Host passes x transposed (xT [1024, 2048]) so the d-contraction
    operands are already partition-major.
  - Q^T/K^T are produced in [channels, t] layout directly (lhsT = W).
  - Scores are computed transposed, S^T[k, q] = (K Q^T)/..., so the
    softmax k-sum can ride the PE: V is augmented with a ones column
    and Y^T_aug = [V|1]^T @ P^T gives the rowsum in row 64.
  - exp has no max-subtraction (scores are ~N(0,1); safe in f32).
  - Causality: per k-tile only the live q range (q >= 128*i) is
    computed; the 128-wide diagonal triangle is masked with a
    precomputed upper-triangular bf16 mask.
  - Each PV matmul is emitted immediately after its exp so P tiles are
    single-read and the PE can run ahead of the (bottleneck) ACT exp.
  - Normalization (divide by rowsum) happens after the PV matmul on
    [64, 512] tiles via a DMA partition-broadcast reciprocal.
"""

from contextlib import ExitStack

import numpy as np
import ml_dtypes

import concourse.bass as bass
import concourse.tile as tile
from concourse import bacc, mybir
from concourse.bass_utils import run_bass_kernel_spmd
from concourse.masks import make_upper_triangular

BF16 = mybir.dt.bfloat16
F32 = mybir.dt.float32
AF = mybir.ActivationFunctionType

T = 2048          # sequence length
D = 1024          # model dim
HD = 64           # head dim
H_LOC = 8         # heads per core
DH = H_LOC * HD   # 512: local qkv width per core
NT = T // 128     # 16 t-tiles
NKD = D // 128    # 8 d k-tiles
NKH = DH // 128   # 4 hd k-tiles
SCALE = 1.0 / np.sqrt(np.float32(HD))  # 0.125


def _make_pools(ctx: ExitStack, tc: tile.TileContext):
    return {
        "persist": ctx.enter_context(tc.tile_pool(name="persist", bufs=1)),
        "ptiles": ctx.enter_context(tc.tile_pool(name="ptiles", bufs=6)),
        "recip": ctx.enter_context(tc.tile_pool(name="recip", bufs=4)),
        "recipb": ctx.enter_context(tc.tile_pool(name="recipb", bufs=4)),
        "outsb": ctx.enter_context(tc.tile_pool(name="outsb", bufs=4)),
        "stg": ctx.enter_context(tc.tile_pool(name="stg", bufs=4)),
        "recipd": ctx.enter_context(tc.tile_pool(name="recipd", bufs=4, space="DRAM")),
        "mmps": ctx.enter_context(tc.tile_pool(name="mmps", bufs=2, space="PSUM")),
        "sps": ctx.enter_context(tc.tile_pool(name="sps", bufs=2, space="PSUM")),
        "ytps": ctx.enter_context(tc.tile_pool(name="ytps", bufs=2, space="PSUM")),
    }


def _build_body(pools: dict, tc: tile.TileContext, io: dict, phase: str = "all"):
    nc = tc.nc
    xt, wq, wk, wv, wo = io["xt"], io["wq"], io["wk"], io["wv"], io["wo"]
    qb, kb, vb, out = io["qb"], io["kb"], io["vb"], io["out"]

    persist = pools["persist"]
    p_pool = pools["ptiles"]
    rc_pool = pools["recip"]
    rb_pool = pools["recipb"]
    stg_pool = pools["stg"]
    ob_pool = pools["outsb"]
    rd_pool = pools["recipd"]
    mm_ps = pools["mmps"]
    s_ps = pools["sps"]
    yt_ps = pools["ytps"]

    # ---- persistent SBUF tensors + loads -------------------------------
    xt_sb = persist.tile([128, NKD, T], BF16)
    nc.sync.dma_start(out=xt_sb, in_=xt.ap().rearrange("(i p) t -> p i t", p=128))
    wq_sb = persist.tile([128, NKD, DH], BF16)
    nc.sync.dma_start(out=wq_sb, in_=wq.ap().rearrange("(i p) n -> p i n", p=128))
    wk_sb = persist.tile([128, NKD, DH], BF16)
    nc.sync.dma_start(out=wk_sb, in_=wk.ap().rearrange("(i p) n -> p i n", p=128))
    wv_sb = persist.tile([128, NKD, DH], BF16)
    nc.sync.dma_start(out=wv_sb, in_=wv.ap().rearrange("(i p) n -> p i n", p=128))
    wo_sb = persist.tile([128, NKH, D], BF16)
    nc.sync.dma_start(out=wo_sb, in_=wo.ap().rearrange("(i p) n -> p i n", p=128))

    qb_sb = persist.tile([128, 4], F32)
    nc.sync.dma_start(out=qb_sb, in_=qb.ap().rearrange("(r p) -> p r", p=128))
    kb_sb = persist.tile([128, 4], F32)
    nc.sync.dma_start(out=kb_sb, in_=kb.ap().rearrange("(r p) -> p r", p=128))
    vb_sb = persist.tile([128, DH], F32)
    vb_ap = vb.ap()
    vb_bcast = bass.AP(tensor=vb_ap.tensor, offset=vb_ap.offset,
                       ap=[[0, 128]] + list(vb_ap.ap))
    nc.gpsimd.dma_start(out=vb_sb, in_=vb_bcast)

    tri = persist.tile([128, 128], BF16)
    make_upper_triangular(nc, tri[:], val=1.0, diag=True)

    # V with a ones column per (t-tile, head): [128, t-tile, head, 65]
    v_aug = persist.tile([128, NT, H_LOC, HD + 1], BF16)
    nc.vector.memset(v_aug[:], 1.0)

    qT_sb = persist.tile([128, 4, T], BF16)   # Q^T: [p, r, t], ch = 128r + p
    kT_sb = persist.tile([128, 4, T], BF16)
    yT_sb = persist.tile([128, NKH, T], BF16)  # Y^T (normalized attention out)

    # ---- V projection (natural layout, per t-tile) ---------------------
    def emit_v_tile(m):
        ps = mm_ps.tile([128, 512], F32, tag="mmps")
        for i in range(NKD):
            nc.tensor.matmul(ps, lhsT=xt_sb[:, i, 128 * m:128 * (m + 1)],
                             rhs=wv_sb[:, i, :],
                             start=(i == 0), stop=(i == NKD - 1))
        nc.vector.tensor_add(
            out=v_aug[:, m, :, 0:HD],
            in0=ps.rearrange("p (h e) -> p h e", e=HD),
            in1=vb_sb.rearrange("p (h e) -> p h e", e=HD),
        )

    # ---- Q^T / K^T projection for one 128-channel row tile r -----------
    def emit_qkT_row(w_sb, b_sb, dst, r):
        for c in range(4):
            ps = mm_ps.tile([128, 512], F32, tag="mmps")
            for i in range(NKD):
                nc.tensor.matmul(ps, lhsT=w_sb[:, i, 128 * r:128 * (r + 1)],
                                 rhs=xt_sb[:, i, 512 * c:512 * (c + 1)],
                                 start=(i == 0), stop=(i == NKD - 1))
            nc.vector.tensor_scalar_add(out=dst[:, r, 512 * c:512 * (c + 1)],
                                        in0=ps, scalar1=b_sb[:, r:r + 1])

    # ---- normalize one [64, 512] chunk of Y^T --------------------------
    # yt (PSUM) is staged to SBUF immediately so the PSUM slot frees fast;
    # the slow DMA-roundtrip broadcast then runs off the critical path.
    def emit_norm(yt, pb, hp, j):
        stg = stg_pool.tile([65, 512], F32, tag="stg")
        nc.vector.tensor_copy(stg, yt)
        rc = rc_pool.tile([1, 512], F32, tag="rc")
        nc.vector.reciprocal(rc, stg[64:65, :])
        # partition-broadcast via DRAM roundtrip (SBUF source APs
        # cannot have a zero partition step; DRAM sources can)
        rd = rd_pool.tile([512], F32, tag="rd")
        nc.sync.dma_start(out=rd, in_=rc)
        rb = rb_pool.tile([64, 512], F32, tag="rb")
        rd_ap = rd[:]
        rd_bcast = bass.AP(tensor=rd_ap.tensor, offset=rd_ap.offset,
                           ap=[[0, 64]] + list(rd_ap.ap))
        nc.sync.dma_start(out=rb, in_=rd_bcast)
        nc.vector.tensor_mul(
            out=yT_sb[pb:pb + 64, hp, 512 * j:512 * (j + 1)],
            in0=stg[0:64, :], in1=rb)

    # ---- attention for one head pair (2*hp, 2*hp+1) --------------------
    # q runs in 1024-wide chunk-pairs J so each exp ACTIVATE covers up
    # to 1024 columns (ACT has ~350 cycles of fixed cost per op).  Each
    # PV matmul is emitted right after its exp (P tiles single-read).
    # Heads are processed sequentially within a chunk-pair to keep the
    # PSUM footprint at 2 yt accumulators.
    def emit_attention_pair(hp):
        heads = [(2 * hp, 0), (2 * hp + 1, 64)]  # (local head, partition base)
        q_of = {h: qT_sb[pb:pb + 64, hp, :] for h, pb in heads}
        k_of = {h: kT_sb[pb:pb + 64, hp, :] for h, pb in heads}
        for J in range(2):  # q chunk-pairs of 1024
            if hp == 0:  # V tiles just in time for the first pair
                for m in range(8 * J, 8 * J + 8):
                    emit_v_tile(m)
            n_k = 8 * J + 8
            for h, pb in heads:
                ytl = yt_ps.tile([65, 512], F32, tag="ytps", name=f"ytl{h}")
                yth = yt_ps.tile([65, 512], F32, tag="ytps", name=f"yth{h}")
                jl, jh = 2 * J, 2 * J + 1

                def emit_y(i, pt, s):
                    # both 512-wide PV accumulations for k-tile i
                    if i <= 4 * jl + 3:
                        qlo = max(512 * jl, s)
                        width = 512 * (jl + 1) - qlo
                        nc.tensor.matmul(ytl[:, qlo - 512 * jl:512],
                                         lhsT=v_aug[:, i, h, :],
                                         rhs=pt[:, qlo - s:qlo - s + width],
                                         start=(i == 0), stop=(i == 4 * jl + 3))
                    qlo = max(512 * jh, s)
                    width = 512 * (jh + 1) - qlo
                    nc.tensor.matmul(yth[:, qlo - 512 * jh:512],
                                     lhsT=v_aug[:, i, h, :],
                                     rhs=pt[:, qlo - s:qlo - s + width],
                                     start=(i == 0), stop=(i == n_k - 1))

                prev = None  # (i, pt, s): PV trails the exp by one k-tile
                for i in range(n_k):
                    s = max(1024 * J, 128 * i)
                    w = 1024 * J + 1024 - s
                    ps = s_ps.tile([128, 1024], F32, tag="sps")
                    for c0 in range(0, w, 512):  # split at the PSUM bank edge
                        cw = min(512, w - c0)
                        nc.tensor.matmul(ps[:, c0:c0 + cw],
                                         lhsT=k_of[h][:, 128 * i:128 * (i + 1)],
                                         rhs=q_of[h][:, s + c0:s + c0 + cw],
                                         start=True, stop=True)
                    pt = p_pool.tile([128, 1024], BF16, tag="pt")
                    nc.scalar.activation(out=pt[:, 0:w], in_=ps[:, 0:w],
                                         func=AF.Exp, scale=float(SCALE))
                    if i >= 8 * J:  # diagonal tile: mask the leading triangle
                        # on GpSimd (idle engine): keep where q >= k, else 0
                        nc.gpsimd.affine_select(
                            out=pt[:, 0:128], in_=pt[:, 0:128],
                            compare_op=mybir.AluOpType.is_ge, fill=0.0,
                            base=0, pattern=[[1, 128]], channel_multiplier=-1)
                    # software pipeline: consume the PREVIOUS tile so the
                    # PE stream never blocks on this iteration's exp
                    if prev is not None:
                        emit_y(*prev)
                    prev = (i, pt, s)
                emit_y(*prev)
                emit_norm(ytl, pb, hp, jl)
                emit_norm(yth, pb, hp, jh)

    # ---- emission order ------------------------------------------------
    if phase == "qkv":  # bench variant: projections only
        for hp in range(4):
            emit_qkT_row(wq_sb, qb_sb, qT_sb, hp)
            emit_qkT_row(wk_sb, kb_sb, kT_sb, hp)
        for m in range(NT):
            emit_v_tile(m)
        scr = rd_pool.tile([128, 96], BF16, tag="scr")
        nc.sync.dma_start(out=scr[:, 0:32], in_=qT_sb[:, 0, 0:32])
        nc.sync.dma_start(out=scr[:, 32:64], in_=kT_sb[:, 0, 0:32])
        nc.sync.dma_start(out=scr[:, 64:96], in_=v_aug[:, 0, 0, 0:32])
        return
    for hp in range(4):
        emit_qkT_row(wq_sb, qb_sb, qT_sb, hp)
        emit_qkT_row(wk_sb, kb_sb, kT_sb, hp)
        emit_attention_pair(hp)
    if phase == "noproj":  # bench variant: skip o_proj
        scr = rd_pool.tile([128, 32], BF16, tag="scr")
        nc.sync.dma_start(out=scr, in_=yT_sb[:, 0, 0:32])
        return

    # ---- o_proj partial: out = Y^T.T @ Wo ------------------------------
    for m in range(NT):
        for c in range(2):
            ps = mm_ps.tile([128, 512], F32, tag="mmps")
            for kt in range(NKH):
                nc.tensor.matmul(ps, lhsT=yT_sb[:, kt, 128 * m:128 * (m + 1)],
                                 rhs=wo_sb[:, kt, 512 * c:512 * (c + 1)],
                                 start=(kt == 0), stop=(kt == NKH - 1))
            ob = ob_pool.tile([128, 512], F32, tag="ob")
            nc.vector.tensor_copy(ob, ps)
            nc.sync.dma_start(out=out.ap()[128 * m:128 * (m + 1),
                                           512 * c:512 * (c + 1)], in_=ob)


def build_nc(loop_reps: int = 1, phase: str = "all"):
    nc = bacc.Bacc("TRN2", target_bir_lowering=False, debug=False, num_devices=8)
    io = {
        "xt": nc.dram_tensor("xt", [D, T], BF16, kind="ExternalInput"),
        "wq": nc.dram_tensor("wq", [D, DH], BF16, kind="ExternalInput"),
        "wk": nc.dram_tensor("wk", [D, DH], BF16, kind="ExternalInput"),
        "wv": nc.dram_tensor("wv", [D, DH], BF16, kind="ExternalInput"),
        "wo": nc.dram_tensor("wo", [DH, D], BF16, kind="ExternalInput"),
        "qb": nc.dram_tensor("qb", [DH], F32, kind="ExternalInput"),
        "kb": nc.dram_tensor("kb", [DH], F32, kind="ExternalInput"),
        "vb": nc.dram_tensor("vb", [DH], F32, kind="ExternalInput"),
        "out": nc.dram_tensor("out", [T, D], F32, kind="ExternalOutput"),
    }
    with tile.TileContext(nc) as tc:
        with ExitStack() as ctx:
            pools = _make_pools(ctx, tc)
            if loop_reps > 1:  # benchmarking build: repeat the body in-NEFF
                with tc.For_i(0, loop_reps, 1):
                    _build_body(pools, tc, io, phase)
            else:
                _build_body(pools, tc, io, phase)
    nc.compile()
    return nc


def make_in_maps(x, qkv_w, qkv_b):
    bf = ml_dtypes.bfloat16
    x = np.asarray(x, np.float32)
    qkv_w = np.asarray(qkv_w, np.float32)
    qkv_b = np.asarray(qkv_b, np.float32)
    in_maps = []
    for c in range(8):
        b, g = divmod(c, 2)
        sl = slice(DH * g, DH * (g + 1))
        in_maps.append({
            "xt": np.ascontiguousarray(x[b].T).astype(bf),
            "wq": np.ascontiguousarray(qkv_w[:, DH * g:DH * (g + 1)]).astype(bf),
            "wk": np.ascontiguousarray(qkv_w[:, D + DH * g:D + DH * (g + 1)]).astype(bf),
            "wv": np.ascontiguousarray(qkv_w[:, 2 * D + DH * g:2 * D + DH * (g + 1)]).astype(bf),
            "wo": None,  # filled by kernel() (needs o_w)
            "qb": np.ascontiguousarray(qkv_b[sl]).astype(np.float32),
            "kb": np.ascontiguousarray(qkv_b[D + DH * g:D + DH * (g + 1)]).astype(np.float32),
            "vb": np.ascontiguousarray(qkv_b[2 * D + DH * g:2 * D + DH * (g + 1)]).astype(np.float32),
        })
    return in_maps


_NC_CACHE = {}


def get_nc():
    if "nc" not in _NC_CACHE:
        _NC_CACHE["nc"] = build_nc()
    return _NC_CACHE["nc"]


def kernel(x, qkv_w, qkv_b, o_w, o_b):
    x = np.asarray(x, np.float32)
    o_w = np.asarray(o_w, np.float32)
    o_b = np.asarray(o_b, np.float32)
    bf = ml_dtypes.bfloat16

    in_maps = make_in_maps(x, qkv_w, qkv_b)
    for c in range(8):
        g = c % 2
        in_maps[c]["wo"] = np.ascontiguousarray(o_w[DH * g:DH * (g + 1), :]).astype(bf)

    nc = get_nc()
    res = run_bass_kernel_spmd(nc, in_maps, core_ids=list(range(8))).results

    out = np.empty((4, T, D), np.float32)
    for b in range(4):
        out[b] = res[2 * b]["out"] + res[2 * b + 1]["out"]
    out += o_b[None, None, :]
    return out



# revision 6
# speedup vs baseline: 1.0249x; 1.0249x over previous
"""Causal self-attention on 8 TRN2 NeuronCores.

Problem: x[4, 2048, 1024], qkv_w[1024, 3072], o_w[1024, 1024] (f32).
Sharding: core c = (batch b = c // 2, head-group g = c % 2 of 8 heads).
Each core computes qkv projection for its (batch, 8 heads), causal
attention, and a partial o_proj ([2048, 1024], f32).  Host sums the two
head-group partials per batch (the "all-reduce") and adds o_b.

Device-side layout choices:
  - All matmuls in bf16 (f32 PSUM accumulate); host pre-casts inputs.
  - Host passes x transposed (xT [1024, 2048]) so the d-contraction
    operands are already partition-major.
  - Q^T/K^T are produced in [channels, t] layout directly (lhsT = W).
  - Scores are computed transposed, S^T[k, q] = (K Q^T)/..., so the
    softmax k-sum can ride the PE: V is augmented with a ones column
    and Y^T_aug = [V|1]^T @ P^T gives the rowsum in row 64.
  - exp has no max-subtraction (scores are ~N(0,1); safe in f32).
  - Causality: per k-tile only the live q range (q >= 128*i) is
    computed; the 128-wide diagonal triangle is masked with a
    precomputed upper-triangular bf16 mask.
  - PV matmuls trail their exp by PIPE_DEPTH k-tiles so the in-order
    PE queue never head-of-line blocks on the (co-critical) ACT exp.
  - Projection/V matmul chains are interleaved one matmul at a time
    into the attention stream ("fillers") to keep the PE continuously
    busy while ACT catches up (avoids HAM throttle oscillation).
  - Normalization: DVE reciprocal of the rowsum row + Pool-engine
    partition_broadcast + DVE multiply (no DMA roundtrip).
  - Input DMAs are spread across engine queues (sync/vector/scalar/
    gpsimd) and x^T is chunked so the first matmul starts early.
"""

from collections import deque
from contextlib import ExitStack

import numpy as np
import ml_dtypes

import concourse.bass as bass
import concourse.tile as tile
from concourse import bacc, mybir
from concourse.bass_utils import run_bass_kernel_spmd
from concourse.masks import make_upper_triangular

BF16 = mybir.dt.bfloat16
F32 = mybir.dt.float32
AF = mybir.ActivationFunctionType

T = 2048          # sequence length
D = 1024          # model dim
HD = 64           # head dim
H_LOC = 8         # heads per core
DH = H_LOC * HD   # 512: local qkv width per core
NT = T // 128     # 16 t-tiles
NKD = D // 128    # 8 d k-tiles
NKH = DH // 128   # 4 hd k-tiles
SCALE = 1.0 / np.sqrt(np.float32(HD))  # 0.125
PIPE_DEPTH = 2    # PV trails exp by this many k-tiles


def _make_pools(ctx: ExitStack, tc: tile.TileContext):
    return {
        "persist": ctx.enter_context(tc.tile_pool(name="persist", bufs=1)),
        "ptiles": ctx.enter_context(tc.tile_pool(name="ptiles", bufs=6)),
        "recip": ctx.enter_context(tc.tile_pool(name="recip", bufs=4)),
        "recipb": ctx.enter_context(tc.tile_pool(name="recipb", bufs=4)),
        "outsb": ctx.enter_context(tc.tile_pool(name="outsb", bufs=4)),
        "scrd": ctx.enter_context(tc.tile_pool(name="scrd", bufs=2, space="DRAM")),
        "mmps": ctx.enter_context(tc.tile_pool(name="mmps", bufs=2, space="PSUM")),
        "sps": ctx.enter_context(tc.tile_pool(name="sps", bufs=2, space="PSUM")),
        "ytps": ctx.enter_context(tc.tile_pool(name="ytps", bufs=2, space="PSUM")),
    }


def _build_body(pools: dict, tc: tile.TileContext, io: dict, phase: str = "all"):
    nc = tc.nc
    xt, wq, wk, wv, wo = io["xt"], io["wq"], io["wk"], io["wv"], io["wo"]
    qb, kb, vb, out = io["qb"], io["kb"], io["vb"], io["out"]

    persist = pools["persist"]
    p_pool = pools["ptiles"]
    rc_pool = pools["recip"]
    rb_pool = pools["recipb"]
    ob_pool = pools["outsb"]
    sc_pool = pools["scrd"]
    mm_ps = pools["mmps"]
    s_ps = pools["sps"]
    yt_ps = pools["ytps"]

    # ---- persistent SBUF tensors + loads -------------------------------
    # Inputs spread across engine DMA queues; xT chunked along t so the
    # first projection matmuls can start after ~1/4 of the load.
    xt_sb = persist.tile([128, NKD, T], BF16)
    for c in range(4):
        nc.sync.dma_start(
            out=xt_sb[:, :, 512 * c:512 * (c + 1)],
            in_=xt.ap()[:, 512 * c:512 * (c + 1)].rearrange(
                "(i p) t -> p i t", p=128))
    wq_sb = persist.tile([128, NKD, DH], BF16)
    nc.scalar.dma_start(out=wq_sb, in_=wq.ap().rearrange("(i p) n -> p i n", p=128))
    wk_sb = persist.tile([128, NKD, DH], BF16)
    nc.scalar.dma_start(out=wk_sb, in_=wk.ap().rearrange("(i p) n -> p i n", p=128))
    wv_sb = persist.tile([128, NKD, DH], BF16)
    nc.gpsimd.dma_start(out=wv_sb, in_=wv.ap().rearrange("(i p) n -> p i n", p=128))
    wo_sb = persist.tile([128, NKH, D], BF16)
    nc.gpsimd.dma_start(out=wo_sb, in_=wo.ap().rearrange("(i p) n -> p i n", p=128))

    qb_sb = persist.tile([128, 4], F32)
    nc.gpsimd.dma_start(out=qb_sb, in_=qb.ap().rearrange("(r p) -> p r", p=128))
    kb_sb = persist.tile([128, 4], F32)
    nc.gpsimd.dma_start(out=kb_sb, in_=kb.ap().rearrange("(r p) -> p r", p=128))
    vb_sb = persist.tile([128, DH], F32)
    vb_ap = vb.ap()
    vb_bcast = bass.AP(tensor=vb_ap.tensor, offset=vb_ap.offset,
                       ap=[[0, 128]] + list(vb_ap.ap))
    nc.gpsimd.dma_start(out=vb_sb, in_=vb_bcast)

    tri = persist.tile([128, 128], BF16)
    make_upper_triangular(nc, tri[:], val=1.0, diag=True)

    # V with a ones column per (t-tile, head): [128, t-tile, head, 65].
    # Only the ones column is memset; the V region is fully overwritten.
    v_aug = persist.tile([128, NT, H_LOC, HD + 1], BF16)
    nc.vector.memset(v_aug[:, :, :, HD:HD + 1], 1.0)

    qT_sb = persist.tile([128, 4, T], BF16)   # Q^T: [p, r, t], ch = 128r + p
    kT_sb = persist.tile([128, 4, T], BF16)
    yT_sb = persist.tile([128, NKH, T], BF16)  # Y^T (normalized attention out)

    # ---- filler machinery ---------------------------------------------
    # Each filler emits ONE independent PE matmul (plus the trailing DVE
    # op on the last matmul of its chain).  Fillers are pumped into the
    # attention stream so the PE never idles while ACT (exp) catches up.
    filler = deque()

    def pump(n):
        for _ in range(n):
            if not filler:
                return
            filler.popleft()()

    def pump_all():
        pump(len(filler))

    def add_proj_chain(w_sb, b_sb, dst, r, c):
        st = {}
        for i in range(NKD):
            def f(i=i, st=st, w_sb=w_sb, b_sb=b_sb, dst=dst, r=r, c=c):
                if i == 0:
                    st["ps"] = mm_ps.tile([128, 512], F32, tag="mmps", name=f"proj{r}_{c}")
                nc.tensor.matmul(st["ps"], lhsT=w_sb[:, i, 128 * r:128 * (r + 1)],
                                 rhs=xt_sb[:, i, 512 * c:512 * (c + 1)],
                                 start=(i == 0), stop=(i == NKD - 1))
                if i == NKD - 1:
                    nc.vector.tensor_scalar_add(
                        out=dst[:, r, 512 * c:512 * (c + 1)],
                        in0=st["ps"], scalar1=b_sb[:, r:r + 1])
            filler.append(f)

    def add_qk_rows(hp):
        for c in range(4):
            add_proj_chain(wq_sb, qb_sb, qT_sb, hp, c)
        for c in range(4):
            add_proj_chain(wk_sb, kb_sb, kT_sb, hp, c)

    def add_v_chain(m):
        st = {}
        for i in range(NKD):
            def f(i=i, st=st, m=m):
                if i == 0:
                    st["ps"] = mm_ps.tile([128, 512], F32, tag="mmps", name=f"vproj{m}")
                nc.tensor.matmul(st["ps"], lhsT=xt_sb[:, i, 128 * m:128 * (m + 1)],
                                 rhs=wv_sb[:, i, :],
                                 start=(i == 0), stop=(i == NKD - 1))
                if i == NKD - 1:
                    nc.vector.tensor_add(
                        out=v_aug[:, m, :, 0:HD],
                        in0=st["ps"].rearrange("p (h e) -> p h e", e=HD),
                        in1=vb_sb.rearrange("p (h e) -> p h e", e=HD))
            filler.append(f)

    # ---- normalize one [64, 512] chunk of Y^T --------------------------
    # recip of the rowsum row (DVE), partition-broadcast (Pool ucode),
    # multiply into yT (DVE).  No DMA roundtrip.
    def emit_norm(yt, pb, hp, j):
        rc = rc_pool.tile([1, 512], F32, tag="rc")
        nc.vector.reciprocal(rc, yt[64:65, :])
        rb = rb_pool.tile([64, 512], F32, tag="rb")
        nc.gpsimd.partition_broadcast(rb[:], rc[:], channels=64)
        nc.vector.tensor_mul(
            out=yT_sb[pb:pb + 64, hp, 512 * j:512 * (j + 1)],
            in0=yt[0:64, :], in1=rb)

    # ---- attention for one head pair (2*hp, 2*hp+1) --------------------
    # q runs in 1024-wide chunk-pairs J so each exp ACTIVATE covers up
    # to 1024 columns (ACT has ~350 cycles of fixed cost per op).
    # Heads are processed sequentially within a chunk-pair to keep the
    # PSUM footprint at 2 yt accumulators.
    def emit_attention_pair(hp):
        heads = [(2 * hp, 0), (2 * hp + 1, 64)]  # (local head, partition base)
        q_of = {h: qT_sb[pb:pb + 64, hp, :] for h, pb in heads}
        k_of = {h: kT_sb[pb:pb + 64, hp, :] for h, pb in heads}
        for J in range(2):  # q chunk-pairs of 1024
            if hp == 0:
                if J == 0:
                    for m in range(8):      # V tiles for J=0: block-emitted
                        add_v_chain(m)
                    pump_all()
                else:
                    for m in range(8, 16):  # V tiles for J=1: ride as fillers
                        add_v_chain(m)      # (ahead of hp1's projections)
                    if hp + 1 < 4:
                        add_qk_rows(hp + 1)
            elif J == 0 and hp + 1 < 4:
                add_qk_rows(hp + 1)         # next head-pair's projections
            n_k = 8 * J + 8
            for hi, (h, pb) in enumerate(heads):
                # pump rate: hp0-J1 head0 needs 4/tile so V chain m=8+k
                # completes before PV(8+k) (verified: done at tile 2k+2
                # <= consume tile 10+k); 2/tile elsewhere.
                rate = 4 if (hp == 0 and J == 1 and hi == 0) else 2
                ytl = yt_ps.tile([65, 512], F32, tag="ytps", name=f"ytl{h}")
                yth = yt_ps.tile([65, 512], F32, tag="ytps", name=f"yth{h}")
                jl, jh = 2 * J, 2 * J + 1

                def emit_y(i, pt, s, ytl=ytl, yth=yth, jl=jl, jh=jh, h=h,
                           n_k=n_k):
                    # both 512-wide PV accumulations for k-tile i
                    if i <= 4 * jl + 3:
                        qlo = max(512 * jl, s)
                        width = 512 * (jl + 1) - qlo
                        nc.tensor.matmul(ytl[:, qlo - 512 * jl:512],
                                         lhsT=v_aug[:, i, h, :],
                                         rhs=pt[:, qlo - s:qlo - s + width],
                                         start=(i == 0), stop=(i == 4 * jl + 3))
                    qlo = max(512 * jh, s)
                    width = 512 * (jh + 1) - qlo
                    nc.tensor.matmul(yth[:, qlo - 512 * jh:512],
                                     lhsT=v_aug[:, i, h, :],
                                     rhs=pt[:, qlo - s:qlo - s + width],
                                     start=(i == 0), stop=(i == n_k - 1))

                pending = deque()  # (i, pt, s): PV trails exp by PIPE_DEPTH
                for i in range(n_k):
                    s = max(1024 * J, 128 * i)
                    w = 1024 * J + 1024 - s
                    ps = s_ps.tile([128, 1024], F32, tag="sps")
                    for c0 in range(0, w, 512):  # split at the PSUM bank edge
                        cw = min(512, w - c0)
                        nc.tensor.matmul(ps[:, c0:c0 + cw],
                                         lhsT=k_of[h][:, 128 * i:128 * (i + 1)],
                                         rhs=q_of[h][:, s + c0:s + c0 + cw],
                                         start=True, stop=True)
                    pt = p_pool.tile([128, 1024], BF16, tag="pt")
                    nc.scalar.activation(out=pt[:, 0:w], in_=ps[:, 0:w],
                                         func=AF.Exp, scale=float(SCALE))
                    if i >= 8 * J:  # diagonal tile: mask the leading triangle
                        # on GpSimd (idle engine): keep where q >= k, else 0
                        nc.gpsimd.affine_select(
                            out=pt[:, 0:128], in_=pt[:, 0:128],
                            compare_op=mybir.AluOpType.is_ge, fill=0.0,
                            base=0, pattern=[[1, 128]], channel_multiplier=-1)
                    if len(pending) >= PIPE_DEPTH:
                        emit_y(*pending.popleft())
                    pending.append((i, pt, s))
                    pump(rate)
                while pending:
                    emit_y(*pending.popleft())
                emit_norm(ytl, pb, hp, jl)
                emit_norm(yth, pb, hp, jh)

    # ---- emission order ------------------------------------------------
    if phase == "qkv":  # bench variant: projections only
        for hp in range(4):
            add_qk_rows(hp)
        for m in range(NT):
            add_v_chain(m)
        pump_all()
        scr = sc_pool.tile([128, 96], BF16, tag="scr")
        nc.sync.dma_start(out=scr[:, 0:32], in_=qT_sb[:, 0, 0:32])
        nc.sync.dma_start(out=scr[:, 32:64], in_=kT_sb[:, 0, 0:32])
        nc.sync.dma_start(out=scr[:, 64:96], in_=v_aug[:, 0, 0, 0:32])
        return

    add_qk_rows(0)
    pump_all()          # hp0's Q/K rows must precede its attention
    for hp in range(4):
        emit_attention_pair(hp)
        pump_all()      # flush: hp+1's projections complete before its scores
    if phase == "noproj":  # bench variant: skip o_proj
        scr = sc_pool.tile([128, 32], BF16, tag="scr")
        nc.sync.dma_start(out=scr, in_=yT_sb[:, 0, 0:32])
        return

    # ---- o_proj partial: out = Y^T.T @ Wo ------------------------------
    for m in range(NT):
        for c in range(2):
            ps = mm_ps.tile([128, 512], F32, tag="mmps")
            for kt in range(NKH):
                nc.tensor.matmul(ps, lhsT=yT_sb[:, kt, 128 * m:128 * (m + 1)],
                                 rhs=wo_sb[:, kt, 512 * c:512 * (c + 1)],
                                 start=(kt == 0), stop=(kt == NKH - 1))
            ob = ob_pool.tile([128, 512], F32, tag="ob")
            nc.vector.tensor_copy(ob, ps)
            nc.gpsimd.dma_start(out=out.ap()[128 * m:128 * (m + 1),
                                             512 * c:512 * (c + 1)], in_=ob)


def build_nc(loop_reps: int = 1, phase: str = "all"):
    nc = bacc.Bacc("TRN2", target_bir_lowering=False, debug=False, num_devices=8)
    io = {
        "xt": nc.dram_tensor("xt", [D, T], BF16, kind="ExternalInput"),
        "wq": nc.dram_tensor("wq", [D, DH], BF16, kind="ExternalInput"),
        "wk": nc.dram_tensor("wk", [D, DH], BF16, kind="ExternalInput"),
        "wv": nc.dram_tensor("wv", [D, DH], BF16, kind="ExternalInput"),
        "wo": nc.dram_tensor("wo", [DH, D], BF16, kind="ExternalInput"),
        "qb": nc.dram_tensor("qb", [DH], F32, kind="ExternalInput"),
        "kb": nc.dram_tensor("kb", [DH], F32, kind="ExternalInput"),
        "vb": nc.dram_tensor("vb", [DH], F32, kind="ExternalInput"),
        "out": nc.dram_tensor("out", [T, D], F32, kind="ExternalOutput"),
    }
    with tile.TileContext(nc) as tc:
        with ExitStack() as ctx:
            pools = _make_pools(ctx, tc)
            if loop_reps > 1:  # benchmarking build: repeat the body in-NEFF
                with tc.For_i(0, loop_reps, 1):
                    _build_body(pools, tc, io, phase)
            else:
                _build_body(pools, tc, io, phase)
    nc.compile()
    return nc


def make_in_maps(x, qkv_w, qkv_b):
    bf = ml_dtypes.bfloat16
    x = np.asarray(x, np.float32)
    qkv_w = np.asarray(qkv_w, np.float32)
    qkv_b = np.asarray(qkv_b, np.float32)
    in_maps = []
    for c in range(8):
        b, g = divmod(c, 2)
        sl = slice(DH * g, DH * (g + 1))
        in_maps.append({
            "xt": np.ascontiguousarray(x[b].T).astype(bf),
            "wq": np.ascontiguousarray(qkv_w[:, DH * g:DH * (g + 1)]).astype(bf),
            "wk": np.ascontiguousarray(qkv_w[:, D + DH * g:D + DH * (g + 1)]).astype(bf),
            "wv": np.ascontiguousarray(qkv_w[:, 2 * D + DH * g:2 * D + DH * (g + 1)]).astype(bf),
            "wo": None,  # filled by kernel() (needs o_w)
            "qb": np.ascontiguousarray(qkv_b[sl]).astype(np.float32),
            "kb": np.ascontiguousarray(qkv_b[D + DH * g:D + DH * (g + 1)]).astype(np.float32),
            "vb": np.ascontiguousarray(qkv_b[2 * D + DH * g:2 * D + DH * (g + 1)]).astype(np.float32),
        })
    return in_maps


_NC_CACHE = {}


def get_nc():
    if "nc" not in _NC_CACHE:
        _NC_CACHE["nc"] = build_nc()
    return _NC_CACHE["nc"]


def kernel(x, qkv_w, qkv_b, o_w, o_b):
    x = np.asarray(x, np.float32)
    o_w = np.asarray(o_w, np.float32)
    o_b = np.asarray(o_b, np.float32)
    bf = ml_dtypes.bfloat16

    in_maps = make_in_maps(x, qkv_w, qkv_b)
    for c in range(8):
        g = c % 2
        in_maps[c]["wo"] = np.ascontiguousarray(o_w[DH * g:DH * (g + 1), :]).astype(bf)

    nc = get_nc()
    res = run_bass_kernel_spmd(nc, in_maps, core_ids=list(range(8))).results

    out = np.empty((4, T, D), np.float32)
    for b in range(4):
        out[b] = res[2 * b]["out"] + res[2 * b + 1]["out"]
    out += o_b[None, None, :]
    return out


# revision 18
# speedup vs baseline: 1.1872x; 1.1584x over previous
"""Causal self-attention on 8 TRN2 NeuronCores.

Problem: x[4, 2048, 1024], qkv_w[1024, 3072], o_w[1024, 1024] (f32).
Sharding: core c = (batch b = c // 2, head-group g = c % 2 of 8 heads).
Each core computes qkv projection for its (batch, 8 heads), causal
attention, and a partial o_proj ([2048, 1024], f32).  Host sums the two
head-group partials per batch (the "all-reduce") and adds o_b.

Device-side layout choices:
  - All matmuls in bf16 (f32 PSUM accumulate); host pre-casts inputs.
  - Host passes x transposed (xT [1024, 2048]) so the d-contraction
    operands are already partition-major.
  - Q^T/K^T are produced in [channels, t] layout directly (lhsT = W).
  - Scores are computed transposed, S^T[k, q] = (K Q^T)/..., so the
    softmax k-sum can ride the PE: V is augmented with a ones column
    and Y^T_aug = [V|1]^T @ P^T gives the rowsum in row 64.
  - exp has no max-subtraction (scores are ~N(0,1); safe in f32).
  - Causality: per k-tile only the live q range (q >= 128*i) is
    computed; the 128-wide diagonal triangle is masked with a
    precomputed upper-triangular bf16 mask.
  - Each PV matmul is emitted immediately after its exp so P tiles are
    single-read and the PE can run ahead of the (bottleneck) ACT exp.
  - Normalization (divide by rowsum) happens after the PV matmul on
    [64, 512] tiles via a DMA partition-broadcast reciprocal.
"""

from contextlib import ExitStack

import numpy as np
import ml_dtypes

import concourse.bass as bass
import concourse.tile as tile
from concourse import bacc, mybir
from concourse.bass_utils import run_bass_kernel_spmd
from concourse.masks import make_upper_triangular

BF16 = mybir.dt.bfloat16
F32 = mybir.dt.float32
AF = mybir.ActivationFunctionType

T = 2048          # sequence length
D = 1024          # model dim
HD = 64           # head dim
H_LOC = 8         # heads per core
DH = H_LOC * HD   # 512: local qkv width per core
NT = T // 128     # 16 t-tiles
NKD = D // 128    # 8 d k-tiles
NKH = DH // 128   # 4 hd k-tiles
SCALE = 1.0 / np.sqrt(np.float32(HD))  # 0.125


def _make_pools(ctx: ExitStack, tc: tile.TileContext):
    return {
        "persist": ctx.enter_context(tc.tile_pool(name="persist", bufs=1)),
        "ptiles": ctx.enter_context(tc.tile_pool(name="ptiles", bufs=6)),
        "recip": ctx.enter_context(tc.tile_pool(name="recip", bufs=4)),
        "recipb": ctx.enter_context(tc.tile_pool(name="recipb", bufs=4)),
        "outsb": ctx.enter_context(tc.tile_pool(name="outsb", bufs=4)),
        "stg": ctx.enter_context(tc.tile_pool(name="stg", bufs=4)),
        "recipd": ctx.enter_context(tc.tile_pool(name="recipd", bufs=4, space="DRAM")),
        "mmps": ctx.enter_context(tc.tile_pool(name="mmps", bufs=2, space="PSUM")),
        "sps": ctx.enter_context(tc.tile_pool(name="sps", bufs=2, space="PSUM")),
        "ytps": ctx.enter_context(tc.tile_pool(name="ytps", bufs=2, space="PSUM")),
    }


def _build_body(pools: dict, tc: tile.TileContext, io: dict, phase: str = "all"):
    nc = tc.nc
    xt, wq, wk, wv, wo = io["xt"], io["wq"], io["wk"], io["wv"], io["wo"]
    qb, kb, vb, out = io["qb"], io["kb"], io["vb"], io["out"]

    persist = pools["persist"]
    p_pool = pools["ptiles"]
    rc_pool = pools["recip"]
    rb_pool = pools["recipb"]
    stg_pool = pools["stg"]
    ob_pool = pools["outsb"]
    rd_pool = pools["recipd"]
    mm_ps = pools["mmps"]
    s_ps = pools["sps"]
    yt_ps = pools["ytps"]

    # ---- persistent SBUF tensors + loads -------------------------------
    xt_sb = persist.tile([128, NKD, T], BF16)
    nc.sync.dma_start(out=xt_sb, in_=xt.ap().rearrange("(i p) t -> p i t", p=128))
    wq_sb = persist.tile([128, NKD, DH], BF16)
    nc.sync.dma_start(out=wq_sb, in_=wq.ap().rearrange("(i p) n -> p i n", p=128))
    wk_sb = persist.tile([128, NKD, DH], BF16)
    nc.sync.dma_start(out=wk_sb, in_=wk.ap().rearrange("(i p) n -> p i n", p=128))
    wv_sb = persist.tile([128, NKD, DH], BF16)
    nc.sync.dma_start(out=wv_sb, in_=wv.ap().rearrange("(i p) n -> p i n", p=128))
    wo_sb = persist.tile([128, NKH, D], BF16)
    nc.sync.dma_start(out=wo_sb, in_=wo.ap().rearrange("(i p) n -> p i n", p=128))

    qb_sb = persist.tile([128, 4], F32)
    nc.sync.dma_start(out=qb_sb, in_=qb.ap().rearrange("(r p) -> p r", p=128))
    kb_sb = persist.tile([128, 4], F32)
    nc.sync.dma_start(out=kb_sb, in_=kb.ap().rearrange("(r p) -> p r", p=128))
    vb_sb = persist.tile([128, DH], F32)
    vb_ap = vb.ap()
    vb_bcast = bass.AP(tensor=vb_ap.tensor, offset=vb_ap.offset,
                       ap=[[0, 128]] + list(vb_ap.ap))
    nc.gpsimd.dma_start(out=vb_sb, in_=vb_bcast)

    tri = persist.tile([128, 128], BF16)
    make_upper_triangular(nc, tri[:], val=1.0, diag=True)

    # V with a ones column per (t-tile, head): [128, t-tile, head, 65]
    v_aug = persist.tile([128, NT, H_LOC, HD + 1], BF16)
    nc.vector.memset(v_aug[:], 1.0)

    qT_sb = persist.tile([128, 4, T], BF16)   # Q^T: [p, r, t], ch = 128r + p
    kT_sb = persist.tile([128, 4, T], BF16)
    yT_sb = persist.tile([128, NKH, T], BF16)  # Y^T (normalized attention out)

    # ---- V projection (natural layout, per t-tile) ---------------------
    def emit_v_tile(m):
        ps = mm_ps.tile([128, 512], F32, tag="mmps")
        for i in range(NKD):
            nc.tensor.matmul(ps, lhsT=xt_sb[:, i, 128 * m:128 * (m + 1)],
                             rhs=wv_sb[:, i, :],
                             start=(i == 0), stop=(i == NKD - 1))
        nc.vector.tensor_add(
            out=v_aug[:, m, :, 0:HD],
            in0=ps.rearrange("p (h e) -> p h e", e=HD),
            in1=vb_sb.rearrange("p (h e) -> p h e", e=HD),
        )

    # ---- Q^T / K^T projection for one 128-channel row tile r -----------
    def emit_qkT_row(w_sb, b_sb, dst, r):
        for c in range(4):
            ps = mm_ps.tile([128, 512], F32, tag="mmps")
            for i in range(NKD):
                nc.tensor.matmul(ps, lhsT=w_sb[:, i, 128 * r:128 * (r + 1)],
                                 rhs=xt_sb[:, i, 512 * c:512 * (c + 1)],
                                 start=(i == 0), stop=(i == NKD - 1))
            nc.vector.tensor_scalar_add(out=dst[:, r, 512 * c:512 * (c + 1)],
                                        in0=ps, scalar1=b_sb[:, r:r + 1])

    # ---- normalize one [64, 512] chunk of Y^T --------------------------
    # yt (PSUM) is staged to SBUF immediately so the PSUM slot frees fast;
    # the slow DMA-roundtrip broadcast then runs off the critical path.
    def emit_norm(yt, pb, hp, j):
        stg = stg_pool.tile([65, 512], F32, tag="stg")
        nc.vector.tensor_copy(stg, yt)
        rc = rc_pool.tile([1, 512], F32, tag="rc")
        nc.vector.reciprocal(rc, stg[64:65, :])
        # partition-broadcast via DRAM roundtrip (SBUF source APs
        # cannot have a zero partition step; DRAM sources can)
        rd = rd_pool.tile([512], F32, tag="rd")
        nc.sync.dma_start(out=rd, in_=rc)
        rb = rb_pool.tile([64, 512], F32, tag="rb")
        rd_ap = rd[:]
        rd_bcast = bass.AP(tensor=rd_ap.tensor, offset=rd_ap.offset,
                           ap=[[0, 64]] + list(rd_ap.ap))
        nc.sync.dma_start(out=rb, in_=rd_bcast)
        nc.vector.tensor_mul(
            out=yT_sb[pb:pb + 64, hp, 512 * j:512 * (j + 1)],
            in0=stg[0:64, :], in1=rb)

    # ---- attention for one head pair (2*hp, 2*hp+1) --------------------
    # q runs in 1024-wide chunk-pairs J so each exp ACTIVATE covers up
    # to 1024 columns (ACT has ~350 cycles of fixed cost per op).  Each
    # PV matmul is emitted right after its exp (P tiles single-read).
    # Heads are processed sequentially within a chunk-pair to keep the
    # PSUM footprint at 2 yt accumulators.
    def emit_attention_pair(hp):
        heads = [(2 * hp, 0), (2 * hp + 1, 64)]  # (local head, partition base)
        q_of = {h: qT_sb[pb:pb + 64, hp, :] for h, pb in heads}
        k_of = {h: kT_sb[pb:pb + 64, hp, :] for h, pb in heads}
        for J in range(2):  # q chunk-pairs of 1024
            if hp == 0:  # V tiles just in time for the first pair
                for m in range(8 * J, 8 * J + 8):
                    emit_v_tile(m)
            n_k = 8 * J + 8
            for h, pb in heads:
                ytl = yt_ps.tile([65, 512], F32, tag="ytps", name=f"ytl{h}")
                yth = yt_ps.tile([65, 512], F32, tag="ytps", name=f"yth{h}")
                jl, jh = 2 * J, 2 * J + 1

                def emit_y(i, pt, s):
                    # both 512-wide PV accumulations for k-tile i
                    if i <= 4 * jl + 3:
                        qlo = max(512 * jl, s)
                        width = 512 * (jl + 1) - qlo
                        nc.tensor.matmul(ytl[:, qlo - 512 * jl:512],
                                         lhsT=v_aug[:, i, h, :],
                                         rhs=pt[:, qlo - s:qlo - s + width],
                                         start=(i == 0), stop=(i == 4 * jl + 3))
                    qlo = max(512 * jh, s)
                    width = 512 * (jh + 1) - qlo
                    nc.tensor.matmul(yth[:, qlo - 512 * jh:512],
                                     lhsT=v_aug[:, i, h, :],
                                     rhs=pt[:, qlo - s:qlo - s + width],
                                     start=(i == 0), stop=(i == n_k - 1))

                prev = None  # (i, pt, s): PV trails the exp by one k-tile
                for i in range(n_k):
                    s = max(1024 * J, 128 * i)
                    w = 1024 * J + 1024 - s
                    ps = s_ps.tile([128, 1024], F32, tag="sps")
                    for c0 in range(0, w, 512):  # split at the PSUM bank edge
                        cw = min(512, w - c0)
                        nc.tensor.matmul(ps[:, c0:c0 + cw],
                                         lhsT=k_of[h][:, 128 * i:128 * (i + 1)],
                                         rhs=q_of[h][:, s + c0:s + c0 + cw],
                                         start=True, stop=True)
                    pt = p_pool.tile([128, 1024], BF16, tag="pt")
                    nc.scalar.activation(out=pt[:, 0:w], in_=ps[:, 0:w],
                                         func=AF.Exp, scale=float(SCALE))
                    if i >= 8 * J:  # diagonal tile: mask the leading triangle
                        # on GpSimd (idle engine): keep where q >= k, else 0
                        nc.gpsimd.affine_select(
                            out=pt[:, 0:128], in_=pt[:, 0:128],
                            compare_op=mybir.AluOpType.is_ge, fill=0.0,
                            base=0, pattern=[[1, 128]], channel_multiplier=-1)
                    # software pipeline: consume the PREVIOUS tile so the
                    # PE stream never blocks on this iteration's exp
                    if prev is not None:
                        emit_y(*prev)
                    prev = (i, pt, s)
                emit_y(*prev)
                emit_norm(ytl, pb, hp, jl)
                emit_norm(yth, pb, hp, jh)

    # ---- emission order ------------------------------------------------
    if phase == "qkv":  # bench variant: projections only
        for hp in range(4):
            emit_qkT_row(wq_sb, qb_sb, qT_sb, hp)
            emit_qkT_row(wk_sb, kb_sb, kT_sb, hp)
        for m in range(NT):
            emit_v_tile(m)
        scr = rd_pool.tile([128, 96], BF16, tag="scr")
        nc.sync.dma_start(out=scr[:, 0:32], in_=qT_sb[:, 0, 0:32])
        nc.sync.dma_start(out=scr[:, 32:64], in_=kT_sb[:, 0, 0:32])
        nc.sync.dma_start(out=scr[:, 64:96], in_=v_aug[:, 0, 0, 0:32])
        return
    for hp in range(4):
        emit_qkT_row(wq_sb, qb_sb, qT_sb, hp)
        emit_qkT_row(wk_sb, kb_sb, kT_sb, hp)
        emit_attention_pair(hp)
    if phase == "noproj":  # bench variant: skip o_proj
        scr = rd_pool.tile([128, 32], BF16, tag="scr")
        nc.sync.dma_start(out=scr, in_=yT_sb[:, 0, 0:32])
        return

    # ---- o_proj partial: out = Y^T.T @ Wo ------------------------------
    for m in range(NT):
        for c in range(2):
            ps = mm_ps.tile([128, 512], F32, tag="mmps")
            for kt in range(NKH):
                nc.tensor.matmul(ps, lhsT=yT_sb[:, kt, 128 * m:128 * (m + 1)],
                                 rhs=wo_sb[:, kt, 512 * c:512 * (c + 1)],
                                 start=(kt == 0), stop=(kt == NKH - 1))
            ob = ob_pool.tile([128, 512], F32, tag="ob")
            nc.vector.tensor_copy(ob, ps)
            nc.sync.dma_start(out=out.ap()[128 * m:128 * (m + 1),
                                           512 * c:512 * (c + 1)], in_=ob)


def build_nc(loop_reps: int = 1, phase: str = "all"):
    nc = bacc.Bacc("TRN2", target_bir_lowering=False, debug=False, num_devices=8)
    io = {
        "xt": nc.dram_tensor("xt", [D, T], BF16, kind="ExternalInput"),
        "wq": nc.dram_tensor("wq", [D, DH], BF16, kind="ExternalInput"),
        "wk": nc.dram_tensor("wk", [D, DH], BF16, kind="ExternalInput"),
        "wv": nc.dram_tensor("wv", [D, DH], BF16, kind="ExternalInput"),
        "wo": nc.dram_tensor("wo", [DH, D], BF16, kind="ExternalInput"),
        "qb": nc.dram_tensor("qb", [DH], F32, kind="ExternalInput"),
        "kb": nc.dram_tensor("kb", [DH], F32, kind="ExternalInput"),
        "vb": nc.dram_tensor("vb", [DH], F32, kind="ExternalInput"),
        "out": nc.dram_tensor("out", [T, D], F32, kind="ExternalOutput"),
    }
    with tile.TileContext(nc) as tc:
        with ExitStack() as ctx:
            pools = _make_pools(ctx, tc)
            if loop_reps > 1:  # benchmarking build: repeat the body in-NEFF
                with tc.For_i(0, loop_reps, 1):
                    _build_body(pools, tc, io, phase)
            else:
                _build_body(pools, tc, io, phase)
    nc.compile()
    return nc


def make_in_maps(x, qkv_w, qkv_b):
    bf = ml_dtypes.bfloat16
    x = np.asarray(x, np.float32)
    qkv_w = np.asarray(qkv_w, np.float32)
    qkv_b = np.asarray(qkv_b, np.float32)
    in_maps = []
    for c in range(8):
        b, g = divmod(c, 2)
        sl = slice(DH * g, DH * (g + 1))
        in_maps.append({
            "xt": np.ascontiguousarray(x[b].T).astype(bf),
            "wq": np.ascontiguousarray(qkv_w[:, DH * g:DH * (g + 1)]).astype(bf),
            "wk": np.ascontiguousarray(qkv_w[:, D + DH * g:D + DH * (g + 1)]).astype(bf),
            "wv": np.ascontiguousarray(qkv_w[:, 2 * D + DH * g:2 * D + DH * (g + 1)]).astype(bf),
            "wo": None,  # filled by kernel() (needs o_w)
            "qb": np.ascontiguousarray(qkv_b[sl]).astype(np.float32),
            "kb": np.ascontiguousarray(qkv_b[D + DH * g:D + DH * (g + 1)]).astype(np.float32),
            "vb": np.ascontiguousarray(qkv_b[2 * D + DH * g:2 * D + DH * (g + 1)]).astype(np.float32),
        })
    return in_maps


_NC_CACHE = {}


def get_nc():
    if "nc" not in _NC_CACHE:
        _NC_CACHE["nc"] = build_nc()
    return _NC_CACHE["nc"]


def kernel(x, qkv_w, qkv_b, o_w, o_b):
    x = np.asarray(x, np.float32)
    o_w = np.asarray(o_w, np.float32)
    o_b = np.asarray(o_b, np.float32)
    bf = ml_dtypes.bfloat16

    in_maps = make_in_maps(x, qkv_w, qkv_b)
    for c in range(8):
        g = c % 2
        in_maps[c]["wo"] = np.ascontiguousarray(o_w[DH * g:DH * (g + 1), :]).astype(bf)

    nc = get_nc()
    res = run_bass_kernel_spmd(nc, in_maps, core_ids=list(range(8))).results

    out = np.empty((4, T, D), np.float32)
    for b in range(4):
        out[b] = res[2 * b]["out"] + res[2 * b + 1]["out"]
    out += o_b[None, None, :]
    return out



# revision 25
# speedup vs baseline: 1.2005x; 1.0111x over previous
"""Causal self-attention on 8 TRN2 NeuronCores.

Problem: x[4, 2048, 1024], qkv_w[1024, 3072], o_w[1024, 1024] (f32).
Sharding: core c = (batch b = c // 2, head-group g = c % 2 of 8 heads).
Each core computes qkv projection for its (batch, 8 heads), causal
attention, and a partial o_proj ([2048, 1024], f32).  Host sums the two
head-group partials per batch (the "all-reduce") and adds o_b.

Device-side layout choices:
  - All matmuls in bf16 (f32 PSUM accumulate); host pre-casts inputs.
  - Host passes x transposed (xT [1024, 2048]) so the d-contraction
    operands are already partition-major.
  - Q^T/K^T are produced in [channels, t] layout directly (lhsT = W).
  - Scores are computed transposed, S^T[k, q] = (K Q^T)/..., so the
    softmax k-sum can ride the PE: V is augmented with a ones column
    and Y^T_aug = [V|1]^T @ P^T gives the rowsum in row 64.
  - exp has no max-subtraction (scores are ~N(0,1); safe in f32).
  - Causality: per k-tile only the live q range (q >= 128*i) is
    computed; the 128-wide diagonal triangle is masked with a
    precomputed upper-triangular bf16 mask.
  - Each PV matmul trails its exp by one k-tile; independent projection
    matmuls ("fillers") are pumped into the PE stream BETWEEN the score
    matmuls and the trailing PV, so the in-order PE queue does useful
    work exactly where it would otherwise stall waiting for the ACT exp
    (the stall point is in front of the PV, not after it).  Fillers:
    V tiles m8-15 + next pair's Q/K rows during hp0-J1/hp1/hp2, and the
    first half of o_proj during hp3-J1 (its yT deps complete at hp3-J0).
  - Normalization (divide by rowsum) happens after the PV matmul on
    [64, 512] tiles via a DMA partition-broadcast reciprocal.
"""

from collections import deque
from contextlib import ExitStack

import numpy as np
import ml_dtypes

import concourse.bass as bass
import concourse.tile as tile
from concourse import bacc, mybir
from concourse.bass_utils import run_bass_kernel_spmd
from concourse.masks import make_upper_triangular

BF16 = mybir.dt.bfloat16
F32 = mybir.dt.float32
AF = mybir.ActivationFunctionType

T = 2048          # sequence length
D = 1024          # model dim
HD = 64           # head dim
H_LOC = 8         # heads per core
DH = H_LOC * HD   # 512: local qkv width per core
NT = T // 128     # 16 t-tiles
NKD = D // 128    # 8 d k-tiles
NKH = DH // 128   # 4 hd k-tiles
SCALE = 1.0 / np.sqrt(np.float32(HD))  # 0.125

import os as _os
FILL_RATE = int(_os.environ.get("BK_RATE", "2"))  # fillers per attention tile


def _make_pools(ctx: ExitStack, tc: tile.TileContext):
    return {
        "persist": ctx.enter_context(tc.tile_pool(name="persist", bufs=1)),
        "ptiles": ctx.enter_context(tc.tile_pool(name="ptiles", bufs=6)),
        "recip": ctx.enter_context(tc.tile_pool(name="recip", bufs=4)),
        "recipb": ctx.enter_context(tc.tile_pool(name="recipb", bufs=4)),
        "outsb": ctx.enter_context(tc.tile_pool(name="outsb", bufs=4)),
        "stg": ctx.enter_context(tc.tile_pool(name="stg", bufs=4)),
        "recipd": ctx.enter_context(tc.tile_pool(name="recipd", bufs=4, space="DRAM")),
        "mmps": ctx.enter_context(tc.tile_pool(name="mmps", bufs=2, space="PSUM")),
        "sps": ctx.enter_context(tc.tile_pool(name="sps", bufs=2, space="PSUM")),
        "ytps": ctx.enter_context(tc.tile_pool(name="ytps", bufs=2, space="PSUM")),
    }


def _build_body(pools: dict, tc: tile.TileContext, io: dict, phase: str = "all"):
    nc = tc.nc
    xt, wq, wk, wv, wo = io["xt"], io["wq"], io["wk"], io["wv"], io["wo"]
    qb, kb, vb, out = io["qb"], io["kb"], io["vb"], io["out"]

    persist = pools["persist"]
    p_pool = pools["ptiles"]
    rc_pool = pools["recip"]
    rb_pool = pools["recipb"]
    stg_pool = pools["stg"]
    ob_pool = pools["outsb"]
    rd_pool = pools["recipd"]
    mm_ps = pools["mmps"]
    s_ps = pools["sps"]
    yt_ps = pools["ytps"]

    # ---- persistent SBUF tensors + loads -------------------------------
    xt_sb = persist.tile([128, NKD, T], BF16)
    nc.sync.dma_start(out=xt_sb, in_=xt.ap().rearrange("(i p) t -> p i t", p=128))
    wq_sb = persist.tile([128, NKD, DH], BF16)
    nc.sync.dma_start(out=wq_sb, in_=wq.ap().rearrange("(i p) n -> p i n", p=128))
    wk_sb = persist.tile([128, NKD, DH], BF16)
    nc.sync.dma_start(out=wk_sb, in_=wk.ap().rearrange("(i p) n -> p i n", p=128))
    wv_sb = persist.tile([128, NKD, DH], BF16)
    nc.sync.dma_start(out=wv_sb, in_=wv.ap().rearrange("(i p) n -> p i n", p=128))
    wo_sb = persist.tile([128, NKH, D], BF16)
    nc.sync.dma_start(out=wo_sb, in_=wo.ap().rearrange("(i p) n -> p i n", p=128))

    qb_sb = persist.tile([128, 4], F32)
    nc.sync.dma_start(out=qb_sb, in_=qb.ap().rearrange("(r p) -> p r", p=128))
    kb_sb = persist.tile([128, 4], F32)
    nc.sync.dma_start(out=kb_sb, in_=kb.ap().rearrange("(r p) -> p r", p=128))
    vb_sb = persist.tile([128, DH], F32)
    vb_ap = vb.ap()
    vb_bcast = bass.AP(tensor=vb_ap.tensor, offset=vb_ap.offset,
                       ap=[[0, 128]] + list(vb_ap.ap))
    nc.gpsimd.dma_start(out=vb_sb, in_=vb_bcast)

    tri = persist.tile([128, 128], BF16)
    make_upper_triangular(nc, tri[:], val=1.0, diag=True)

    # V with a ones column per (t-tile, head): [128, t-tile, head, 65]
    v_aug = persist.tile([128, NT, H_LOC, HD + 1], BF16)
    nc.vector.memset(v_aug[:], 1.0)

    qT_sb = persist.tile([128, 4, T], BF16)   # Q^T: [p, r, t], ch = 128r + p
    kT_sb = persist.tile([128, 4, T], BF16)
    yT_sb = persist.tile([128, NKH, T], BF16)  # Y^T (normalized attention out)

    # ---- filler machinery ---------------------------------------------
    # Each filler emits ONE independent PE matmul (plus a trailing DVE op
    # on its chain's last matmul).  Fillers are pumped into the PE stream
    # between the score matmuls and the trailing PV of each attention
    # k-tile — the exact spot where the in-order PE queue would stall
    # waiting for the exp.
    filler = deque()

    def pump(n):
        for _ in range(n):
            if not filler:
                return
            filler.popleft()()

    def pump_all():
        pump(len(filler))

    def add_v_tile(m):
        st = {}
        for i in range(NKD):
            def f(i=i, st=st, m=m):
                if i == 0:
                    st["ps"] = mm_ps.tile([128, 512], F32, tag="mmps",
                                          name=f"vps{m}")
                nc.tensor.matmul(st["ps"], lhsT=xt_sb[:, i, 128 * m:128 * (m + 1)],
                                 rhs=wv_sb[:, i, :],
                                 start=(i == 0), stop=(i == NKD - 1))
                if i == NKD - 1:
                    nc.vector.tensor_add(
                        out=v_aug[:, m, :, 0:HD],
                        in0=st["ps"].rearrange("p (h e) -> p h e", e=HD),
                        in1=vb_sb.rearrange("p (h e) -> p h e", e=HD))
            filler.append(f)

    def add_qkT_row(w_sb, b_sb, dst, r):
        for c in range(4):
            st = {}
            for i in range(NKD):
                def f(i=i, st=st, w_sb=w_sb, b_sb=b_sb, dst=dst, r=r, c=c):
                    if i == 0:
                        st["ps"] = mm_ps.tile([128, 512], F32, tag="mmps",
                                              name=f"qkps{r}_{c}")
                    nc.tensor.matmul(st["ps"],
                                     lhsT=w_sb[:, i, 128 * r:128 * (r + 1)],
                                     rhs=xt_sb[:, i, 512 * c:512 * (c + 1)],
                                     start=(i == 0), stop=(i == NKD - 1))
                    if i == NKD - 1:
                        nc.vector.tensor_scalar_add(
                            out=dst[:, r, 512 * c:512 * (c + 1)],
                            in0=st["ps"], scalar1=b_sb[:, r:r + 1])
                filler.append(f)

    def add_qk_rows(hp):
        add_qkT_row(wq_sb, qb_sb, qT_sb, hp)
        add_qkT_row(wk_sb, kb_sb, kT_sb, hp)

    def add_oproj_chunk(m, c):
        st = {}
        for kt in range(NKH):
            def f(kt=kt, st=st, m=m, c=c):
                if kt == 0:
                    st["ps"] = mm_ps.tile([128, 512], F32, tag="mmps",
                                          name=f"ops{m}_{c}")
                nc.tensor.matmul(st["ps"],
                                 lhsT=yT_sb[:, kt, 128 * m:128 * (m + 1)],
                                 rhs=wo_sb[:, kt, 512 * c:512 * (c + 1)],
                                 start=(kt == 0), stop=(kt == NKH - 1))
                if kt == NKH - 1:
                    ob = ob_pool.tile([128, 512], F32, tag="ob",
                                      name=f"ob{m}_{c}")
                    nc.vector.tensor_copy(ob, st["ps"])
                    nc.sync.dma_start(
                        out=out.ap()[128 * m:128 * (m + 1),
                                     512 * c:512 * (c + 1)], in_=ob)
            filler.append(f)

    # ---- normalize one [64, 512] chunk of Y^T --------------------------
    # yt (PSUM) is staged to SBUF immediately so the PSUM slot frees fast;
    # the slow DMA-roundtrip broadcast then runs off the critical path.
    def emit_norm(yt, pb, hp, j):
        stg = stg_pool.tile([65, 512], F32, tag="stg")
        nc.vector.tensor_copy(stg, yt)
        rc = rc_pool.tile([1, 512], F32, tag="rc")
        nc.vector.reciprocal(rc, stg[64:65, :])
        # partition-broadcast via DRAM roundtrip (SBUF source APs
        # cannot have a zero partition step; DRAM sources can)
        rd = rd_pool.tile([512], F32, tag="rd")
        nc.sync.dma_start(out=rd, in_=rc)
        rb = rb_pool.tile([64, 512], F32, tag="rb")
        rd_ap = rd[:]
        rd_bcast = bass.AP(tensor=rd_ap.tensor, offset=rd_ap.offset,
                           ap=[[0, 64]] + list(rd_ap.ap))
        nc.sync.dma_start(out=rb, in_=rd_bcast)
        nc.vector.tensor_mul(
            out=yT_sb[pb:pb + 64, hp, 512 * j:512 * (j + 1)],
            in0=stg[0:64, :], in1=rb)

    # ---- attention for one head pair (2*hp, 2*hp+1) --------------------
    # q runs in 1024-wide chunk-pairs J so each exp ACTIVATE covers up
    # to 1024 columns (ACT has ~350 cycles of fixed cost per op).
    # Heads are processed sequentially within a chunk-pair to keep the
    # PSUM footprint at 2 yt accumulators.
    def emit_attention_pair(hp):
        heads = [(2 * hp, 0), (2 * hp + 1, 64)]  # (local head, partition base)
        q_of = {h: qT_sb[pb:pb + 64, hp, :] for h, pb in heads}
        k_of = {h: kT_sb[pb:pb + 64, hp, :] for h, pb in heads}
        for J in range(2):  # q chunk-pairs of 1024
            if hp == 0:
                # V tiles are consumed by THIS pair's PVs: emitting them as
                # rate-limited fillers reorders a write after its reader,
                # which the program-order dep tracking reads as "use the old
                # value" (silently wrong).  Block-emit them instead; only
                # hazard-free work (consumed after a flush) rides as filler.
                for m in range(8 * J, 8 * J + 8):
                    add_v_tile(m)
                pump_all()
                if J == 1:
                    add_qk_rows(1)
            elif J == 0 and hp + 1 < 4:
                add_qk_rows(hp + 1)
            elif J == 1 and hp == 3:
                # hp3-J0's norms completed yT[:, 3, 0:1024]; rows 0-2 are
                # long done -> first half of o_proj can fill hp3-J1.
                for m in range(8):
                    for c in range(2):
                        add_oproj_chunk(m, c)
            n_k = 8 * J + 8
            for h, pb in heads:
                ytl = yt_ps.tile([65, 512], F32, tag="ytps", name=f"ytl{h}")
                yth = yt_ps.tile([65, 512], F32, tag="ytps", name=f"yth{h}")
                jl, jh = 2 * J, 2 * J + 1

                def emit_y(i, pt, s, ytl=ytl, yth=yth, jl=jl, jh=jh, h=h,
                           n_k=n_k):
                    # both 512-wide PV accumulations for k-tile i
                    if i <= 4 * jl + 3:
                        qlo = max(512 * jl, s)
                        width = 512 * (jl + 1) - qlo
                        nc.tensor.matmul(ytl[:, qlo - 512 * jl:512],
                                         lhsT=v_aug[:, i, h, :],
                                         rhs=pt[:, qlo - s:qlo - s + width],
                                         start=(i == 0), stop=(i == 4 * jl + 3))
                    qlo = max(512 * jh, s)
                    width = 512 * (jh + 1) - qlo
                    nc.tensor.matmul(yth[:, qlo - 512 * jh:512],
                                     lhsT=v_aug[:, i, h, :],
                                     rhs=pt[:, qlo - s:qlo - s + width],
                                     start=(i == 0), stop=(i == n_k - 1))

                prev = None  # (i, pt, s): PV trails the exp by one k-tile
                for i in range(n_k):
                    s = max(1024 * J, 128 * i)
                    w = 1024 * J + 1024 - s
                    ps = s_ps.tile([128, 1024], F32, tag="sps")
                    for c0 in range(0, w, 512):  # split at the PSUM bank edge
                        cw = min(512, w - c0)
                        nc.tensor.matmul(ps[:, c0:c0 + cw],
                                         lhsT=k_of[h][:, 128 * i:128 * (i + 1)],
                                         rhs=q_of[h][:, s + c0:s + c0 + cw],
                                         start=True, stop=True)
                    pt = p_pool.tile([128, 1024], BF16, tag="pt")
                    nc.scalar.activation(out=pt[:, 0:w], in_=ps[:, 0:w],
                                         func=AF.Exp, scale=float(SCALE))
                    if i >= 8 * J:  # diagonal tile: mask the leading triangle
                        # on GpSimd (idle engine): keep where q >= k, else 0
                        nc.gpsimd.affine_select(
                            out=pt[:, 0:128], in_=pt[:, 0:128],
                            compare_op=mybir.AluOpType.is_ge, fill=0.0,
                            base=0, pattern=[[1, 128]], channel_multiplier=-1)
                    # fillers BEFORE the trailing PV: they occupy the PE
                    # exactly while it would wait for exp(i-1) to finish
                    pump(FILL_RATE)
                    if prev is not None:
                        emit_y(*prev)
                    prev = (i, pt, s)
                emit_y(*prev)
                emit_norm(ytl, pb, hp, jl)
                emit_norm(yth, pb, hp, jh)

    # ---- emission order ------------------------------------------------
    if phase == "qkv":  # bench variant: projections only
        for hp in range(4):
            add_qk_rows(hp)
        for m in range(NT):
            add_v_tile(m)
        pump_all()
        scr = rd_pool.tile([128, 96], BF16, tag="scr")
        nc.sync.dma_start(out=scr[:, 0:32], in_=qT_sb[:, 0, 0:32])
        nc.sync.dma_start(out=scr[:, 32:64], in_=kT_sb[:, 0, 0:32])
        nc.sync.dma_start(out=scr[:, 64:96], in_=v_aug[:, 0, 0, 0:32])
        return

    add_qk_rows(0)
    pump_all()          # hp0's Q/K rows must precede its attention
    for hp in range(4):
        emit_attention_pair(hp)
        pump_all()      # flush: hp+1's Q/K rows complete before its scores
    if phase == "noproj":  # bench variant: skip o_proj
        scr = rd_pool.tile([128, 32], BF16, tag="scr")
        nc.sync.dma_start(out=scr, in_=yT_sb[:, 0, 0:32])
        return

    # ---- o_proj second half (first half rode as hp3-J1 fillers) --------
    for m in range(8, NT):
        for c in range(2):
            add_oproj_chunk(m, c)
    pump_all()


def build_nc(loop_reps: int = 1, phase: str = "all"):
    nc = bacc.Bacc("TRN2", target_bir_lowering=False, debug=False, num_devices=8)
    io = {
        "xt": nc.dram_tensor("xt", [D, T], BF16, kind="ExternalInput"),
        "wq": nc.dram_tensor("wq", [D, DH], BF16, kind="ExternalInput"),
        "wk": nc.dram_tensor("wk", [D, DH], BF16, kind="ExternalInput"),
        "wv": nc.dram_tensor("wv", [D, DH], BF16, kind="ExternalInput"),
        "wo": nc.dram_tensor("wo", [DH, D], BF16, kind="ExternalInput"),
        "qb": nc.dram_tensor("qb", [DH], F32, kind="ExternalInput"),
        "kb": nc.dram_tensor("kb", [DH], F32, kind="ExternalInput"),
        "vb": nc.dram_tensor("vb", [DH], F32, kind="ExternalInput"),
        "out": nc.dram_tensor("out", [T, D], F32, kind="ExternalOutput"),
    }
    with tile.TileContext(nc) as tc:
        with ExitStack() as ctx:
            pools = _make_pools(ctx, tc)
            if loop_reps > 1:  # benchmarking build: repeat the body in-NEFF
                with tc.For_i(0, loop_reps, 1):
                    _build_body(pools, tc, io, phase)
            else:
                _build_body(pools, tc, io, phase)
    nc.compile()
    return nc


def make_in_maps(x, qkv_w, qkv_b):
    bf = ml_dtypes.bfloat16
    x = np.asarray(x, np.float32)
    qkv_w = np.asarray(qkv_w, np.float32)
    qkv_b = np.asarray(qkv_b, np.float32)
    in_maps = []
    for c in range(8):
        b, g = divmod(c, 2)
        sl = slice(DH * g, DH * (g + 1))
        in_maps.append({
            "xt": np.ascontiguousarray(x[b].T).astype(bf),
            "wq": np.ascontiguousarray(qkv_w[:, DH * g:DH * (g + 1)]).astype(bf),
            "wk": np.ascontiguousarray(qkv_w[:, D + DH * g:D + DH * (g + 1)]).astype(bf),
            "wv": np.ascontiguousarray(qkv_w[:, 2 * D + DH * g:2 * D + DH * (g + 1)]).astype(bf),
            "wo": None,  # filled by kernel() (needs o_w)
            "qb": np.ascontiguousarray(qkv_b[sl]).astype(np.float32),
            "kb": np.ascontiguousarray(qkv_b[D + DH * g:D + DH * (g + 1)]).astype(np.float32),
            "vb": np.ascontiguousarray(qkv_b[2 * D + DH * g:2 * D + DH * (g + 1)]).astype(np.float32),
        })
    return in_maps


_NC_CACHE = {}


def get_nc():
    if "nc" not in _NC_CACHE:
        _NC_CACHE["nc"] = build_nc()
    return _NC_CACHE["nc"]


def kernel(x, qkv_w, qkv_b, o_w, o_b):
    x = np.asarray(x, np.float32)
    o_w = np.asarray(o_w, np.float32)
    o_b = np.asarray(o_b, np.float32)
    bf = ml_dtypes.bfloat16

    in_maps = make_in_maps(x, qkv_w, qkv_b)
    for c in range(8):
        g = c % 2
        in_maps[c]["wo"] = np.ascontiguousarray(o_w[DH * g:DH * (g + 1), :]).astype(bf)

    nc = get_nc()
    res = run_bass_kernel_spmd(nc, in_maps, core_ids=list(range(8))).results

    out = np.empty((4, T, D), np.float32)
    for b in range(4):
        out[b] = res[2 * b]["out"] + res[2 * b + 1]["out"]
    out += o_b[None, None, :]
    return out
